# revision 2
# baseline (speedup 1.0000x reference)
"""Trainium2 Bass kernel: DeepseekV4 CSA Compressor.

Math (per batch b):
  kv = hidden @ w_kv, gate = hidden @ w_gate          [S, 256]
  windows w = 0..S/32-1: tokens [w*32-32, w*32+32)  (prev block -> lo
  channels, current block -> hi channels; window 0 prev = 0 kv / -1e9 gate)
  pooled[w] = sum_j softmax_j(win_g + pos_bias)[j, d] * win_kv[j, d]
  RoPE on trailing 64 dims at position w*32.

Sharding: 8 cores = (4 batches) x (2 sequence halves).  Each core gets its
4096-token chunk transposed on host ([H, 4128] with a 32-token halo column
block in front; zeros for the first half, so the -1e9 gate fill is applied
via a per-core bias variant on the first window group).  No collectives.

Matmuls run in float32r (fp32 with 11-bit mantissa, TF32-like, 4x faster
than fp32 on the PE).  Inputs are pre-rounded to f32r on host.
"""

import numpy as np

HEAD_DIM = 128
ROPE_DIM = 64
RATIO = 32
ROPE_THETA = 10000.0
NEG = -1e9

B, S, H = 4, 8192, 4096
N_CORES = 8
HALF = S // 2                 # tokens per core
NWIN_CORE = HALF // RATIO     # windows per core = 128
GW = 512                      # tokens per matmul/pooling group
WPG = GW // RATIO             # windows per group = 16

_CACHE: dict = {}


def _round_f32r(x: np.ndarray) -> np.ndarray:
    """Round fp32 to f32r (11-bit mantissa, round-to-nearest-even)."""
    u = np.ascontiguousarray(x, dtype=np.float32).view(np.uint32)
    r = (u + np.uint32(0x7FF) + ((u >> np.uint32(12)) & np.uint32(1))) & np.uint32(
        0xFFFFF000
    )
    return r.view(np.float32)


def build_program(T_main: int, H_: int, nwin: int):
    """Build the single-core SPMD Bass program.

    T_main: tokens per core (multiple of GW); H_: hidden dim (multiple of
    128); nwin: windows per core (= T_main // RATIO, multiple of WPG).
    """
    from contextlib import ExitStack

    import concourse.bacc as bacc
    import concourse.mybir as mybir
    import concourse.tile as tile

    f32 = mybir.dt.float32
    f32r = mybir.dt.float32r
    AF = mybir.ActivationFunctionType
    AX = mybir.AxisListType

    d = HEAD_DIM
    r = RATIO
    TT = T_main + r           # with halo block in front
    NG = T_main // GW         # number of groups
    KT = H_ // 128            # k tiles
    C = 4 * d                 # 512 projection channels (kv_lo|kv_hi|g_lo|g_hi)

    nc = bacc.Bacc("TRN2", target_bir_lowering=False, debug=False,
                   num_devices=N_CORES)
    hT = nc.dram_tensor("hT", [H_, TT], f32r, kind="ExternalInput").ap()
    Wt = nc.dram_tensor("W", [H_, C], f32r, kind="ExternalInput").ap()
    bias_lo = nc.dram_tensor("bias_lo", [d, GW], f32, kind="ExternalInput").ap()
    bias_lo0 = nc.dram_tensor("bias_lo0", [d, GW], f32, kind="ExternalInput").ap()
    bias_hi = nc.dram_tensor("bias_hi", [d, GW], f32, kind="ExternalInput").ap()
    cos_in = nc.dram_tensor("cos", [nwin, ROPE_DIM // 2], f32,
                            kind="ExternalInput").ap()
    sin_in = nc.dram_tensor("sin", [nwin, ROPE_DIM // 2], f32,
                            kind="ExternalInput").ap()
    ident = nc.dram_tensor("ident", [d, d], f32, kind="ExternalInput").ap()
    out = nc.dram_tensor("out", [nwin, d], f32, kind="ExternalOutput").ap()

    with tile.TileContext(nc) as tc, ExitStack() as ctx:
        wp = ctx.enter_context(tc.tile_pool(name="wp", bufs=1))
        hp = ctx.enter_context(tc.tile_pool(name="hp", bufs=4))
        pp = ctx.enter_context(tc.tile_pool(name="pp", bufs=2, space="PSUM"))
        sp = ctx.enter_context(tc.tile_pool(name="sp", bufs=2))
        smp = ctx.enter_context(tc.tile_pool(name="smp", bufs=2))
        cp = ctx.enter_context(tc.tile_pool(name="cp", bufs=1))

        # Stationary weights: [128, KT, 512]; k-tile k, channel c is
        # w_sb[:, k, ct*128:(ct+1)*128] (ct: 0=kv_lo 1=kv_hi 2=g_lo 3=g_hi).
        w_sb = wp.tile([128, KT, C], f32r)
        for k in range(KT):
            nc.sync.dma_start(w_sb[:, k, :], Wt[k * 128:(k + 1) * 128, :])

        blo = cp.tile([d, GW], f32, tag="blo")
        nc.sync.dma_start(blo[:], bias_lo[:])
        blo0 = cp.tile([d, GW], f32, tag="blo0")
        nc.sync.dma_start(blo0[:], bias_lo0[:])
        bhi = cp.tile([d, GW], f32, tag="bhi")
        nc.sync.dma_start(bhi[:], bias_hi[:])

        pooled = cp.tile([d, nwin], f32, tag="pooled")

        for g in range(NG):
            kvlo = pp.tile([d, GW], f32, tag="kvlo")
            kvhi = pp.tile([d, GW], f32, tag="kvhi")
            glo = pp.tile([d, GW], f32, tag="glo")
            ghi = pp.tile([d, GW], f32, tag="ghi")
            for k in range(KT):
                ht_k = hp.tile([128, GW + r], f32r, tag="ht")
                nc.sync.dma_start(
                    ht_k[:], hT[k * 128:(k + 1) * 128, g * GW:g * GW + GW + r])
                rlo = ht_k[:, 0:GW]
                rhi = ht_k[:, r:GW + r]
                st, sp_ = (k == 0), (k == KT - 1)
                nc.tensor.matmul(kvlo[:], w_sb[:, k, 0 * d:1 * d], rlo,
                                 start=st, stop=sp_)
                nc.tensor.matmul(kvhi[:], w_sb[:, k, 1 * d:2 * d], rhi,
                                 start=st, stop=sp_)
                nc.tensor.matmul(glo[:], w_sb[:, k, 2 * d:3 * d], rlo,
                                 start=st, stop=sp_)
                nc.tensor.matmul(ghi[:], w_sb[:, k, 3 * d:4 * d], rhi,
                                 start=st, stop=sp_)

            # Softmax-gated pooling for the 16 windows of this group.
            # No max-subtraction: gate values are O(5), exp is safe, and
            # the -1e9 first-window fill underflows exp to exactly 0.
            tglo = sp.tile([d, GW], f32, tag="tglo")
            nc.vector.tensor_add(tglo[:], glo[:], (blo0 if g == 0 else blo)[:])
            tghi = sp.tile([d, GW], f32, tag="tghi")
            nc.vector.tensor_add(tghi[:], ghi[:], bhi[:])
            elo = sp.tile([d, GW], f32, tag="elo")
            nc.scalar.activation(elo[:], tglo[:], AF.Exp)
            ehi = sp.tile([d, GW], f32, tag="ehi")
            nc.scalar.activation(ehi[:], tghi[:], AF.Exp)

            s_lo = smp.tile([d, WPG], f32, tag="slo")
            nc.vector.reduce_sum(
                s_lo[:], elo[:].rearrange("p (w j) -> p w j", j=r), axis=AX.X)
            s_hi = smp.tile([d, WPG], f32, tag="shi")
            nc.vector.reduce_sum(
                s_hi[:], ehi[:].rearrange("p (w j) -> p w j", j=r), axis=AX.X)
            s_all = smp.tile([d, WPG], f32, tag="sall")
            nc.vector.tensor_add(s_all[:], s_lo[:], s_hi[:])

            plo = sp.tile([d, GW], f32, tag="plo")
            nc.vector.tensor_mul(plo[:], elo[:], kvlo[:])
            phi = sp.tile([d, GW], f32, tag="phi")
            nc.vector.tensor_mul(phi[:], ehi[:], kvhi[:])
            n_lo = smp.tile([d, WPG], f32, tag="nlo")
            nc.vector.reduce_sum(
                n_lo[:], plo[:].rearrange("p (w j) -> p w j", j=r), axis=AX.X)
            n_hi = smp.tile([d, WPG], f32, tag="nhi")
            nc.vector.reduce_sum(
                n_hi[:], phi[:].rearrange("p (w j) -> p w j", j=r), axis=AX.X)
            num = smp.tile([d, WPG], f32, tag="num")
            nc.vector.tensor_add(num[:], n_lo[:], n_hi[:])

            rs = smp.tile([d, WPG], f32, tag="rs")
            nc.vector.reciprocal(rs[:], s_all[:])
            nc.vector.tensor_mul(pooled[:, g * WPG:(g + 1) * WPG], num[:], rs[:])

        # Transpose pooled [d, nwin] -> [nwin, d] via PE, then RoPE.
        idt = cp.tile([d, d], f32, tag="idt")
        nc.sync.dma_start(idt[:], ident[:])
        ptr = pp.tile([nwin, d], f32, tag="kvlo")  # reuse a psum slot
        nc.tensor.transpose(ptr[:], pooled[:], idt[:])

        cosb = cp.tile([nwin, ROPE_DIM // 2], f32, tag="cosb")
        nc.sync.dma_start(cosb[:], cos_in[:])
        sinb = cp.tile([nwin, ROPE_DIM // 2], f32, tag="sinb")
        nc.sync.dma_start(sinb[:], sin_in[:])

        outsb = cp.tile([nwin, d], f32, tag="outsb")
        nope_w = d - ROPE_DIM
        nc.vector.tensor_copy(outsb[:, 0:nope_w], ptr[:, 0:nope_w])
        rp = ptr[:, nope_w:d].rearrange("p (a two) -> p a two", two=2)
        re_, ro_ = rp[:, :, 0], rp[:, :, 1]
        op = outsb[:, nope_w:d].rearrange("p (a two) -> p a two", two=2)
        oe_, oo_ = op[:, :, 0], op[:, :, 1]
        hw_ = ROPE_DIM // 2
        t1 = smp.tile([nwin, hw_], f32, tag="t1")
        t2 = smp.tile([nwin, hw_], f32, tag="t2")
        nc.vector.tensor_mul(t1[:], re_, cosb[:])
        nc.vector.tensor_mul(t2[:], ro_, sinb[:])
        nc.vector.tensor_sub(oe_, t1[:], t2[:])
        t3 = smp.tile([nwin, hw_], f32, tag="t3")
        t4 = smp.tile([nwin, hw_], f32, tag="t4")
        nc.vector.tensor_mul(t3[:], ro_, cosb[:])
        nc.vector.tensor_mul(t4[:], re_, sinb[:])
        nc.vector.tensor_add(oo_, t3[:], t4[:])

        nc.sync.dma_start(out[:], outsb[:])

    nc.compile()
    return nc


def _host_inputs(hidden_states, w_kv, w_gate, position_bias,
                 T_main: int, nwin: int, n_cores: int):
    """Build per-core input maps (list of dicts) for the SPMD program."""
    d, r = HEAD_DIM, RATIO
    H_ = hidden_states.shape[2]
    n_total = nwin * n_cores // hidden_states.shape[0]  # windows per batch

    Wfull = np.concatenate([np.asarray(w_kv, np.float32),
                            np.asarray(w_gate, np.float32)], axis=1)
    Wr = _round_f32r(Wfull)

    biasT = np.ascontiguousarray(np.asarray(position_bias, np.float32).T)  # [d, 2r]
    bias_lo_t = np.ascontiguousarray(np.tile(biasT[:, :r], (1, WPG)))
    bias_hi_t = np.ascontiguousarray(np.tile(biasT[:, r:], (1, WPG)))
    bias_lo_g0 = bias_lo_t.copy()
    bias_lo_g0[:, :r] = NEG

    positions = np.arange(n_total, dtype=np.float32) * r
    inv_freq = 1.0 / (ROPE_THETA ** (
        np.arange(0, ROPE_DIM, 2, dtype=np.float32) / ROPE_DIM))
    freqs = positions[:, None] * inv_freq[None, :]         # [n_total, 32]
    cos = np.cos(freqs).astype(np.float32)
    sin = np.sin(freqs).astype(np.float32)
    ident = np.eye(d, dtype=np.float32)

    hs = np.asarray(hidden_states, np.float32)
    halves_per_batch = n_cores // hs.shape[0]
    in_maps = []
    for c in range(n_cores):
        b, hf = c // halves_per_batch, c % halves_per_batch
        start = hf * T_main
        chunk = np.empty((H_, T_main + r), np.float32)
        chunk[:, r:] = hs[b, start:start + T_main].T
        if hf == 0:
            chunk[:, :r] = 0.0
        else:
            chunk[:, :r] = hs[b, start - r:start].T
        w0 = hf * nwin
        in_maps.append({
            "hT": _round_f32r(chunk),
            "W": Wr,
            "bias_lo": bias_lo_t,
            "bias_lo0": bias_lo_g0 if hf == 0 else bias_lo_t,
            "bias_hi": bias_hi_t,
            "cos": np.ascontiguousarray(cos[w0:w0 + nwin]),
            "sin": np.ascontiguousarray(sin[w0:w0 + nwin]),
            "ident": ident,
        })
    return in_maps


def kernel(hidden_states, w_kv, w_gate, position_bias, _want_profile=False):
    """Full-input, full-output entry point.  Shards over 8 NeuronCores."""
    from concourse.bass_utils import run_bass_kernel_spmd

    hs = np.asarray(hidden_states, np.float32)
    B_, S_, H_ = hs.shape
    n = S_ // RATIO
    if "nc" not in _CACHE:
        _CACHE["nc"] = build_program(HALF, H_, NWIN_CORE)
    nc = _CACHE["nc"]

    in_maps = _host_inputs(hs, w_kv, w_gate, position_bias,
                           HALF, NWIN_CORE, N_CORES)
    kwargs = {}
    if _want_profile:
        import os

        os.makedirs("work/prof", exist_ok=True)
        kwargs = {"trace": True, "tmpdir": os.path.abspath("work/prof")}
    res = run_bass_kernel_spmd(nc, in_maps, list(range(N_CORES)), **kwargs)

    out = np.empty((B_, n, HEAD_DIM), np.float32)
    halves_per_batch = N_CORES // B_
    for c in range(N_CORES):
        b, hf = c // halves_per_batch, c % halves_per_batch
        out[b, hf * NWIN_CORE:(hf + 1) * NWIN_CORE] = res.results[c]["out"]
    if _want_profile:
        return out, res
    return out


# revision 4
# speedup vs baseline: 1.2795x; 1.2795x over previous
"""Trainium2 Bass kernel: DeepseekV4 CSA Compressor.

Math (per batch b):
  kv = hidden @ w_kv, gate = hidden @ w_gate          [S, 256]
  windows w = 0..S/32-1: tokens [w*32-32, w*32+32)  (prev block -> lo
  channels, current block -> hi channels; window 0 prev = 0 kv / -1e9 gate)
  pooled[w] = sum_j softmax_j(win_g + pos_bias)[j, d] * win_kv[j, d]
  RoPE on trailing 64 dims at position w*32.

Sharding: 8 cores = (4 batches) x (2 sequence halves).  Each core gets its
4096-token chunk transposed on host ([H, 4128] with a 32-token halo column
block in front; zeros for the first half, so the -1e9 gate fill is applied
via a per-core bias variant on the first window group).  No collectives.

Matmuls run in float32r (fp32 with 11-bit mantissa, TF32-like, 4x faster
than fp32 on the PE).  Inputs are pre-rounded to f32r on host.
"""

import numpy as np

HEAD_DIM = 128
ROPE_DIM = 64
RATIO = 32
ROPE_THETA = 10000.0
NEG = -1e9

B, S, H = 4, 8192, 4096
N_CORES = 8
HALF = S // 2                 # tokens per core
NWIN_CORE = HALF // RATIO     # windows per core = 128
GW = 512                      # tokens per matmul/pooling group
WPG = GW // RATIO             # windows per group = 16

_CACHE: dict = {}


def _round_f32r(x: np.ndarray) -> np.ndarray:
    """Round fp32 to f32r (11-bit mantissa, round-to-nearest-even)."""
    u = np.ascontiguousarray(x, dtype=np.float32).view(np.uint32)
    r = (u + np.uint32(0x7FF) + ((u >> np.uint32(12)) & np.uint32(1))) & np.uint32(
        0xFFFFF000
    )
    return r.view(np.float32)


def build_program(T_main: int, H_: int, nwin: int):
    """Build the single-core SPMD Bass program.

    T_main: tokens per core (multiple of GW); H_: hidden dim (multiple of
    128); nwin: windows per core (= T_main // RATIO, multiple of WPG).
    """
    from contextlib import ExitStack

    import concourse.bacc as bacc
    import concourse.mybir as mybir
    import concourse.tile as tile

    f32 = mybir.dt.float32
    f32r = mybir.dt.float32r
    AF = mybir.ActivationFunctionType
    AX = mybir.AxisListType

    d = HEAD_DIM
    r = RATIO
    TT = T_main + r           # with halo block in front
    NG = T_main // GW         # number of groups
    KT = H_ // 128            # k tiles
    C = 4 * d                 # 512 projection channels (kv_lo|kv_hi|g_lo|g_hi)

    nc = bacc.Bacc("TRN2", target_bir_lowering=False, debug=False,
                   num_devices=N_CORES)
    hT = nc.dram_tensor("hT", [H_, TT], f32r, kind="ExternalInput").ap()
    Wt = nc.dram_tensor("W", [H_, C], f32r, kind="ExternalInput").ap()
    bias_lo = nc.dram_tensor("bias_lo", [d, GW], f32, kind="ExternalInput").ap()
    bias_lo0 = nc.dram_tensor("bias_lo0", [d, GW], f32, kind="ExternalInput").ap()
    bias_hi = nc.dram_tensor("bias_hi", [d, GW], f32, kind="ExternalInput").ap()
    cos_in = nc.dram_tensor("cos", [nwin, ROPE_DIM // 2], f32,
                            kind="ExternalInput").ap()
    sin_in = nc.dram_tensor("sin", [nwin, ROPE_DIM // 2], f32,
                            kind="ExternalInput").ap()
    ident = nc.dram_tensor("ident", [d, d], f32, kind="ExternalInput").ap()
    out = nc.dram_tensor("out", [nwin, d], f32, kind="ExternalOutput").ap()

    with tile.TileContext(nc) as tc, ExitStack() as ctx:
        wp = ctx.enter_context(tc.tile_pool(name="wp", bufs=1))
        hp = ctx.enter_context(tc.tile_pool(name="hp", bufs=4))
        pp = ctx.enter_context(tc.tile_pool(name="pp", bufs=2, space="PSUM"))
        sp = ctx.enter_context(tc.tile_pool(name="sp", bufs=2))
        smp = ctx.enter_context(tc.tile_pool(name="smp", bufs=2))
        cp = ctx.enter_context(tc.tile_pool(name="cp", bufs=1))

        # Stationary weights: [128, KT, 512]; k-tile k, channel c is
        # w_sb[:, k, ct*128:(ct+1)*128] (ct: 0=kv_lo 1=kv_hi 2=g_lo 3=g_hi).
        # Small/one-time DMAs go on gpsimd (SWDGE) to keep the two HWDGE
        # engines (sync, scalar) free for the hidden-state stream.
        w_sb = wp.tile([128, KT, C], f32r)
        for k in range(KT):
            nc.gpsimd.dma_start(w_sb[:, k, :], Wt[k * 128:(k + 1) * 128, :])

        blo = cp.tile([d, GW], f32, tag="blo")
        nc.gpsimd.dma_start(blo[:], bias_lo[:])
        blo0 = cp.tile([d, GW], f32, tag="blo0")
        nc.gpsimd.dma_start(blo0[:], bias_lo0[:])
        bhi = cp.tile([d, GW], f32, tag="bhi")
        nc.gpsimd.dma_start(bhi[:], bias_hi[:])

        pooled = cp.tile([d, nwin], f32, tag="pooled")

        def pooling(g, kvlo, kvhi, glo, ghi):
            # Softmax-gated pooling for the 16 windows of group g.
            # No max-subtraction: gate values are O(5), exp is safe, and
            # the -1e9 first-window fill underflows exp to exactly 0.
            tglo = sp.tile([d, GW], f32, tag="tglo")
            nc.vector.tensor_add(tglo[:], glo[:], (blo0 if g == 0 else blo)[:])
            tghi = sp.tile([d, GW], f32, tag="tghi")
            nc.vector.tensor_add(tghi[:], ghi[:], bhi[:])
            elo = sp.tile([d, GW], f32, tag="elo")
            nc.scalar.activation(elo[:], tglo[:], AF.Exp)
            ehi = sp.tile([d, GW], f32, tag="ehi")
            nc.scalar.activation(ehi[:], tghi[:], AF.Exp)

            s_lo = smp.tile([d, WPG], f32, tag="slo")
            nc.vector.reduce_sum(
                s_lo[:], elo[:].rearrange("p (w j) -> p w j", j=r), axis=AX.X)
            s_hi = smp.tile([d, WPG], f32, tag="shi")
            nc.vector.reduce_sum(
                s_hi[:], ehi[:].rearrange("p (w j) -> p w j", j=r), axis=AX.X)
            s_all = smp.tile([d, WPG], f32, tag="sall")
            nc.vector.tensor_add(s_all[:], s_lo[:], s_hi[:])

            plo = sp.tile([d, GW], f32, tag="plo")
            nc.vector.tensor_mul(plo[:], elo[:], kvlo[:])
            phi = sp.tile([d, GW], f32, tag="phi")
            nc.vector.tensor_mul(phi[:], ehi[:], kvhi[:])
            n_lo = smp.tile([d, WPG], f32, tag="nlo")
            nc.vector.reduce_sum(
                n_lo[:], plo[:].rearrange("p (w j) -> p w j", j=r), axis=AX.X)
            n_hi = smp.tile([d, WPG], f32, tag="nhi")
            nc.vector.reduce_sum(
                n_hi[:], phi[:].rearrange("p (w j) -> p w j", j=r), axis=AX.X)
            num = smp.tile([d, WPG], f32, tag="num")
            nc.vector.tensor_add(num[:], n_lo[:], n_hi[:])

            rs = smp.tile([d, WPG], f32, tag="rs")
            nc.vector.reciprocal(rs[:], s_all[:])
            nc.vector.tensor_mul(pooled[:, g * WPG:(g + 1) * WPG], num[:], rs[:])

        # Groups processed in pairs: one hT DMA per k-tile covers both
        # groups' (overlapping, 32-shifted) column spans -> 4224B DMA
        # lines and half the descriptor-generation work; the weight tile
        # w_sb[:, k, c] feeds both groups' matmuls.
        assert NG % 2 == 0
        PW = 2 * GW + r  # 1056 columns per pair load
        for p in range(NG // 2):
            g0, g1 = 2 * p, 2 * p + 1
            ps0 = [pp.tile([d, GW], f32, tag=t, name=f"{t}_a{p}")
                   for t in ("kvlo", "kvhi", "glo", "ghi")]
            ps1 = [pp.tile([d, GW], f32, tag=t, name=f"{t}_b{p}")
                   for t in ("kvlo", "kvhi", "glo", "ghi")]
            for k in range(KT):
                ht_k = hp.tile([128, PW], f32r, tag="ht")
                dma_eng = nc.sync if k % 2 == 0 else nc.scalar
                dma_eng.dma_start(
                    ht_k[:], hT[k * 128:(k + 1) * 128, p * 2 * GW:p * 2 * GW + PW])
                views = [
                    (ps0[0], ht_k[:, 0:GW], 0),
                    (ps0[1], ht_k[:, r:GW + r], 1),
                    (ps0[2], ht_k[:, 0:GW], 2),
                    (ps0[3], ht_k[:, r:GW + r], 3),
                    (ps1[0], ht_k[:, GW:2 * GW], 0),
                    (ps1[1], ht_k[:, GW + r:2 * GW + r], 1),
                    (ps1[2], ht_k[:, GW:2 * GW], 2),
                    (ps1[3], ht_k[:, GW + r:2 * GW + r], 3),
                ]
                st, sp_ = (k == 0), (k == KT - 1)
                for psum_t, rhs_v, ct in views:
                    nc.tensor.matmul(psum_t[:], w_sb[:, k, ct * d:(ct + 1) * d],
                                     rhs_v, start=st, stop=sp_)
            pooling(g0, *ps0)
            pooling(g1, *ps1)

        # Transpose pooled [d, nwin] -> [nwin, d] via PE, then RoPE.
        idt = cp.tile([d, d], f32, tag="idt")
        nc.sync.dma_start(idt[:], ident[:])
        ptr = pp.tile([nwin, d], f32, tag="kvlo")  # reuse a psum slot
        nc.tensor.transpose(ptr[:], pooled[:], idt[:])

        cosb = cp.tile([nwin, ROPE_DIM // 2], f32, tag="cosb")
        nc.sync.dma_start(cosb[:], cos_in[:])
        sinb = cp.tile([nwin, ROPE_DIM // 2], f32, tag="sinb")
        nc.sync.dma_start(sinb[:], sin_in[:])

        outsb = cp.tile([nwin, d], f32, tag="outsb")
        nope_w = d - ROPE_DIM
        nc.vector.tensor_copy(outsb[:, 0:nope_w], ptr[:, 0:nope_w])
        rp = ptr[:, nope_w:d].rearrange("p (a two) -> p a two", two=2)
        re_, ro_ = rp[:, :, 0], rp[:, :, 1]
        op = outsb[:, nope_w:d].rearrange("p (a two) -> p a two", two=2)
        oe_, oo_ = op[:, :, 0], op[:, :, 1]
        hw_ = ROPE_DIM // 2
        t1 = smp.tile([nwin, hw_], f32, tag="t1")
        t2 = smp.tile([nwin, hw_], f32, tag="t2")
        nc.vector.tensor_mul(t1[:], re_, cosb[:])
        nc.vector.tensor_mul(t2[:], ro_, sinb[:])
        nc.vector.tensor_sub(oe_, t1[:], t2[:])
        t3 = smp.tile([nwin, hw_], f32, tag="t3")
        t4 = smp.tile([nwin, hw_], f32, tag="t4")
        nc.vector.tensor_mul(t3[:], ro_, cosb[:])
        nc.vector.tensor_mul(t4[:], re_, sinb[:])
        nc.vector.tensor_add(oo_, t3[:], t4[:])

        nc.sync.dma_start(out[:], outsb[:])

    nc.compile()
    return nc


def _host_inputs(hidden_states, w_kv, w_gate, position_bias,
                 T_main: int, nwin: int, n_cores: int):
    """Build per-core input maps (list of dicts) for the SPMD program."""
    d, r = HEAD_DIM, RATIO
    H_ = hidden_states.shape[2]
    n_total = nwin * n_cores // hidden_states.shape[0]  # windows per batch

    Wfull = np.concatenate([np.asarray(w_kv, np.float32),
                            np.asarray(w_gate, np.float32)], axis=1)
    Wr = _round_f32r(Wfull)

    biasT = np.ascontiguousarray(np.asarray(position_bias, np.float32).T)  # [d, 2r]
    bias_lo_t = np.ascontiguousarray(np.tile(biasT[:, :r], (1, WPG)))
    bias_hi_t = np.ascontiguousarray(np.tile(biasT[:, r:], (1, WPG)))
    bias_lo_g0 = bias_lo_t.copy()
    bias_lo_g0[:, :r] = NEG

    positions = np.arange(n_total, dtype=np.float32) * r
    inv_freq = 1.0 / (ROPE_THETA ** (
        np.arange(0, ROPE_DIM, 2, dtype=np.float32) / ROPE_DIM))
    freqs = positions[:, None] * inv_freq[None, :]         # [n_total, 32]
    cos = np.cos(freqs).astype(np.float32)
    sin = np.sin(freqs).astype(np.float32)
    ident = np.eye(d, dtype=np.float32)

    hs = np.asarray(hidden_states, np.float32)
    halves_per_batch = n_cores // hs.shape[0]
    in_maps = []
    for c in range(n_cores):
        b, hf = c // halves_per_batch, c % halves_per_batch
        start = hf * T_main
        chunk = np.empty((H_, T_main + r), np.float32)
        chunk[:, r:] = hs[b, start:start + T_main].T
        if hf == 0:
            chunk[:, :r] = 0.0
        else:
            chunk[:, :r] = hs[b, start - r:start].T
        w0 = hf * nwin
        in_maps.append({
            "hT": _round_f32r(chunk),
            "W": Wr,
            "bias_lo": bias_lo_t,
            "bias_lo0": bias_lo_g0 if hf == 0 else bias_lo_t,
            "bias_hi": bias_hi_t,
            "cos": np.ascontiguousarray(cos[w0:w0 + nwin]),
            "sin": np.ascontiguousarray(sin[w0:w0 + nwin]),
            "ident": ident,
        })
    return in_maps


def kernel(hidden_states, w_kv, w_gate, position_bias, _want_profile=False):
    """Full-input, full-output entry point.  Shards over 8 NeuronCores."""
    from concourse.bass_utils import run_bass_kernel_spmd

    hs = np.asarray(hidden_states, np.float32)
    B_, S_, H_ = hs.shape
    n = S_ // RATIO
    if "nc" not in _CACHE:
        _CACHE["nc"] = build_program(HALF, H_, NWIN_CORE)
    nc = _CACHE["nc"]

    in_maps = _host_inputs(hs, w_kv, w_gate, position_bias,
                           HALF, NWIN_CORE, N_CORES)
    kwargs = {}
    if _want_profile:
        import os

        os.makedirs("work/prof", exist_ok=True)
        kwargs = {"trace": True, "tmpdir": os.path.abspath("work/prof")}
    res = run_bass_kernel_spmd(nc, in_maps, list(range(N_CORES)), **kwargs)

    out = np.empty((B_, n, HEAD_DIM), np.float32)
    halves_per_batch = N_CORES // B_
    for c in range(N_CORES):
        b, hf = c // halves_per_batch, c % halves_per_batch
        out[b, hf * NWIN_CORE:(hf + 1) * NWIN_CORE] = res.results[c]["out"]
    if _want_profile:
        return out, res
    return out


# revision 6
# speedup vs baseline: 1.4842x; 1.1599x over previous
"""Trainium2 Bass kernel: DeepseekV4 CSA Compressor.

Math (per batch b):
  kv = hidden @ w_kv, gate = hidden @ w_gate          [S, 256]
  windows w = 0..S/32-1: tokens [w*32-32, w*32+32)  (prev block -> lo
  channels, current block -> hi channels; window 0 prev = 0 kv / -1e9 gate)
  pooled[w] = sum_j softmax_j(win_g + pos_bias)[j, d] * win_kv[j, d]
  RoPE on trailing 64 dims at position w*32.

Sharding: 8 cores = (4 batches) x (2 sequence halves).  Each core gets its
4096-token chunk transposed on host ([H, 4128] with a 32-token halo column
block in front; zeros for the first half, so the -1e9 gate fill is applied
via a per-core bias variant on the first window group).  No collectives.

Matmuls run in float32r (fp32 with 11-bit mantissa, TF32-like, 4x faster
than fp32 on the PE).  Inputs are pre-rounded to f32r on host.
"""

import numpy as np

HEAD_DIM = 128
ROPE_DIM = 64
RATIO = 32
ROPE_THETA = 10000.0
NEG = -1e9

B, S, H = 4, 8192, 4096
N_CORES = 8
HALF = S // 2                 # tokens per core
NWIN_CORE = HALF // RATIO     # windows per core = 128
GW = 512                      # tokens per matmul/pooling group
WPG = GW // RATIO             # windows per group = 16

_CACHE: dict = {}


def _round_f32r(x: np.ndarray) -> np.ndarray:
    """Round fp32 to f32r (11-bit mantissa, round-to-nearest-even)."""
    u = np.ascontiguousarray(x, dtype=np.float32).view(np.uint32)
    r = (u + np.uint32(0x7FF) + ((u >> np.uint32(12)) & np.uint32(1))) & np.uint32(
        0xFFFFF000
    )
    return r.view(np.float32)


def build_program(T_main: int, H_: int, nwin: int):
    """Build the single-core SPMD Bass program.

    T_main: tokens per core (multiple of GW); H_: hidden dim (multiple of
    128); nwin: windows per core (= T_main // RATIO, multiple of WPG).
    """
    from contextlib import ExitStack

    import concourse.bacc as bacc
    import concourse.mybir as mybir
    import concourse.tile as tile

    f32 = mybir.dt.float32
    f32r = mybir.dt.float32r
    AF = mybir.ActivationFunctionType
    AX = mybir.AxisListType

    d = HEAD_DIM
    r = RATIO
    TT = T_main + r           # with halo block in front
    NG = T_main // GW         # number of groups
    KT = H_ // 128            # k tiles
    C = 4 * d                 # 512 projection channels (kv_lo|kv_hi|g_lo|g_hi)

    nc = bacc.Bacc("TRN2", target_bir_lowering=False, debug=False,
                   num_devices=N_CORES)
    hT = nc.dram_tensor("hT", [H_, TT], f32r, kind="ExternalInput").ap()
    Wt = nc.dram_tensor("W", [H_, C], f32r, kind="ExternalInput").ap()
    bias_lo = nc.dram_tensor("bias_lo", [d, GW], f32, kind="ExternalInput").ap()
    bias_lo0 = nc.dram_tensor("bias_lo0", [d, GW], f32, kind="ExternalInput").ap()
    bias_hi = nc.dram_tensor("bias_hi", [d, GW], f32, kind="ExternalInput").ap()
    cos_in = nc.dram_tensor("cos", [nwin, ROPE_DIM // 2], f32,
                            kind="ExternalInput").ap()
    sin_in = nc.dram_tensor("sin", [nwin, ROPE_DIM // 2], f32,
                            kind="ExternalInput").ap()
    ident = nc.dram_tensor("ident", [d, d], f32, kind="ExternalInput").ap()
    out = nc.dram_tensor("out", [nwin, d], f32, kind="ExternalOutput").ap()

    with tile.TileContext(nc) as tc, ExitStack() as ctx:
        wp = ctx.enter_context(tc.tile_pool(name="wp", bufs=1))
        hp = ctx.enter_context(tc.tile_pool(name="hp", bufs=4))
        pp = ctx.enter_context(tc.tile_pool(name="pp", bufs=2, space="PSUM"))
        sp = ctx.enter_context(tc.tile_pool(name="sp", bufs=2))
        smp = ctx.enter_context(tc.tile_pool(name="smp", bufs=2))
        cp = ctx.enter_context(tc.tile_pool(name="cp", bufs=1))

        # Stationary weights: [128, KT, 512]; k-tile k, channel c is
        # w_sb[:, k, ct*128:(ct+1)*128] (ct: 0=kv_lo 1=kv_hi 2=g_lo 3=g_hi).
        # Small/one-time DMAs go on gpsimd (SWDGE) to keep the two HWDGE
        # engines (sync, scalar) free for the hidden-state stream.
        w_sb = wp.tile([128, KT, C], f32r)
        for k in range(KT):
            nc.gpsimd.dma_start(w_sb[:, k, :], Wt[k * 128:(k + 1) * 128, :])

        blo = cp.tile([d, GW], f32, tag="blo")
        nc.gpsimd.dma_start(blo[:], bias_lo[:])
        blo0 = cp.tile([d, GW], f32, tag="blo0")
        nc.gpsimd.dma_start(blo0[:], bias_lo0[:])
        bhi = cp.tile([d, GW], f32, tag="bhi")
        nc.gpsimd.dma_start(bhi[:], bias_hi[:])

        pooled = cp.tile([d, nwin], f32, tag="pooled")

        def pooling(g, kvlo, kvhi, glo, ghi):
            # Softmax-gated pooling for the 16 windows of group g.
            # No max-subtraction: gate values are O(5), exp is safe, and
            # the -1e9 first-window fill underflows exp to exactly 0.
            # kv banks are copied out first so the PSUM slots free early
            # (the next pair's matmuls wait on them).
            kvlo_sb = sp.tile([d, GW], f32, tag="kvlo_sb")
            nc.vector.tensor_copy(kvlo_sb[:], kvlo[:])
            kvhi_sb = sp.tile([d, GW], f32, tag="kvhi_sb")
            nc.vector.tensor_copy(kvhi_sb[:], kvhi[:])
            kvlo, kvhi = kvlo_sb, kvhi_sb
            tglo = sp.tile([d, GW], f32, tag="tglo")
            nc.vector.tensor_add(tglo[:], glo[:], (blo0 if g == 0 else blo)[:])
            tghi = sp.tile([d, GW], f32, tag="tghi")
            nc.vector.tensor_add(tghi[:], ghi[:], bhi[:])
            elo = sp.tile([d, GW], f32, tag="elo")
            nc.scalar.activation(elo[:], tglo[:], AF.Exp)
            ehi = sp.tile([d, GW], f32, tag="ehi")
            nc.scalar.activation(ehi[:], tghi[:], AF.Exp)

            s_lo = smp.tile([d, WPG], f32, tag="slo")
            nc.vector.reduce_sum(
                s_lo[:], elo[:].rearrange("p (w j) -> p w j", j=r), axis=AX.X)
            s_hi = smp.tile([d, WPG], f32, tag="shi")
            nc.vector.reduce_sum(
                s_hi[:], ehi[:].rearrange("p (w j) -> p w j", j=r), axis=AX.X)
            s_all = smp.tile([d, WPG], f32, tag="sall")
            nc.vector.tensor_add(s_all[:], s_lo[:], s_hi[:])

            plo = sp.tile([d, GW], f32, tag="plo")
            nc.vector.tensor_mul(plo[:], elo[:], kvlo[:])
            phi = sp.tile([d, GW], f32, tag="phi")
            nc.vector.tensor_mul(phi[:], ehi[:], kvhi[:])
            n_lo = smp.tile([d, WPG], f32, tag="nlo")
            nc.vector.reduce_sum(
                n_lo[:], plo[:].rearrange("p (w j) -> p w j", j=r), axis=AX.X)
            n_hi = smp.tile([d, WPG], f32, tag="nhi")
            nc.vector.reduce_sum(
                n_hi[:], phi[:].rearrange("p (w j) -> p w j", j=r), axis=AX.X)
            num = smp.tile([d, WPG], f32, tag="num")
            nc.vector.tensor_add(num[:], n_lo[:], n_hi[:])

            rs = smp.tile([d, WPG], f32, tag="rs")
            nc.vector.reciprocal(rs[:], s_all[:])
            nc.vector.tensor_mul(pooled[:, g * WPG:(g + 1) * WPG], num[:], rs[:])

        # Groups processed in pairs: one hT DMA per k-tile covers both
        # groups' (overlapping, 32-shifted) column spans -> 4224B DMA
        # lines and half the descriptor-generation work; the weight tile
        # w_sb[:, k, c] feeds both groups' matmuls.
        assert NG % 2 == 0 and KT % 2 == 0
        PW = 2 * GW + r  # 1056 columns per pair load
        for p in range(NG // 2):
            g0, g1 = 2 * p, 2 * p + 1
            c0 = p * 2 * GW
            ps0 = [pp.tile([d, GW], f32, tag=t, name=f"{t}_a{p}")
                   for t in ("kvlo", "kvhi", "glo", "ghi")]
            ps1 = [pp.tile([d, GW], f32, tag=t, name=f"{t}_b{p}")
                   for t in ("kvlo", "kvhi", "glo", "ghi")]
            for kk in range(KT // 2):
                # One 1.08MB DMA covers 2 k-tiles x both groups of the pair.
                ht2 = hp.tile([128, 2, PW], f32r, tag="ht")
                dma_eng = nc.sync if kk % 2 == 0 else nc.scalar
                dma_eng.dma_start(
                    ht2[:],
                    hT[2 * kk * 128:(2 * kk + 2) * 128, c0:c0 + PW].rearrange(
                        "(two p) c -> p two c", p=128))
                for j in range(2):
                    k = 2 * kk + j
                    ht_k = ht2[:, j, :]
                    views = [
                        (ps0[0], ht_k[:, 0:GW], 0),
                        (ps0[1], ht_k[:, r:GW + r], 1),
                        (ps0[2], ht_k[:, 0:GW], 2),
                        (ps0[3], ht_k[:, r:GW + r], 3),
                        (ps1[0], ht_k[:, GW:2 * GW], 0),
                        (ps1[1], ht_k[:, GW + r:2 * GW + r], 1),
                        (ps1[2], ht_k[:, GW:2 * GW], 2),
                        (ps1[3], ht_k[:, GW + r:2 * GW + r], 3),
                    ]
                    st, sp_ = (k == 0), (k == KT - 1)
                    for psum_t, rhs_v, ct in views:
                        nc.tensor.matmul(psum_t[:],
                                         w_sb[:, k, ct * d:(ct + 1) * d],
                                         rhs_v, start=st, stop=sp_)
            pooling(g0, *ps0)
            pooling(g1, *ps1)

        # Transpose pooled [d, nwin] -> [nwin, d] via PE, then RoPE.
        idt = cp.tile([d, d], f32, tag="idt")
        nc.sync.dma_start(idt[:], ident[:])
        ptr = pp.tile([nwin, d], f32, tag="kvlo")  # reuse a psum slot
        nc.tensor.transpose(ptr[:], pooled[:], idt[:])

        cosb = cp.tile([nwin, ROPE_DIM // 2], f32, tag="cosb")
        nc.sync.dma_start(cosb[:], cos_in[:])
        sinb = cp.tile([nwin, ROPE_DIM // 2], f32, tag="sinb")
        nc.sync.dma_start(sinb[:], sin_in[:])

        outsb = cp.tile([nwin, d], f32, tag="outsb")
        nope_w = d - ROPE_DIM
        nc.vector.tensor_copy(outsb[:, 0:nope_w], ptr[:, 0:nope_w])
        rp = ptr[:, nope_w:d].rearrange("p (a two) -> p a two", two=2)
        re_, ro_ = rp[:, :, 0], rp[:, :, 1]
        op = outsb[:, nope_w:d].rearrange("p (a two) -> p a two", two=2)
        oe_, oo_ = op[:, :, 0], op[:, :, 1]
        hw_ = ROPE_DIM // 2
        t1 = smp.tile([nwin, hw_], f32, tag="t1")
        t2 = smp.tile([nwin, hw_], f32, tag="t2")
        nc.vector.tensor_mul(t1[:], re_, cosb[:])
        nc.vector.tensor_mul(t2[:], ro_, sinb[:])
        nc.vector.tensor_sub(oe_, t1[:], t2[:])
        t3 = smp.tile([nwin, hw_], f32, tag="t3")
        t4 = smp.tile([nwin, hw_], f32, tag="t4")
        nc.vector.tensor_mul(t3[:], ro_, cosb[:])
        nc.vector.tensor_mul(t4[:], re_, sinb[:])
        nc.vector.tensor_add(oo_, t3[:], t4[:])

        nc.sync.dma_start(out[:], outsb[:])

    nc.compile()
    return nc


def _host_inputs(hidden_states, w_kv, w_gate, position_bias,
                 T_main: int, nwin: int, n_cores: int):
    """Build per-core input maps (list of dicts) for the SPMD program."""
    d, r = HEAD_DIM, RATIO
    H_ = hidden_states.shape[2]
    n_total = nwin * n_cores // hidden_states.shape[0]  # windows per batch

    Wfull = np.concatenate([np.asarray(w_kv, np.float32),
                            np.asarray(w_gate, np.float32)], axis=1)
    Wr = _round_f32r(Wfull)

    biasT = np.ascontiguousarray(np.asarray(position_bias, np.float32).T)  # [d, 2r]
    bias_lo_t = np.ascontiguousarray(np.tile(biasT[:, :r], (1, WPG)))
    bias_hi_t = np.ascontiguousarray(np.tile(biasT[:, r:], (1, WPG)))
    bias_lo_g0 = bias_lo_t.copy()
    bias_lo_g0[:, :r] = NEG

    positions = np.arange(n_total, dtype=np.float32) * r
    inv_freq = 1.0 / (ROPE_THETA ** (
        np.arange(0, ROPE_DIM, 2, dtype=np.float32) / ROPE_DIM))
    freqs = positions[:, None] * inv_freq[None, :]         # [n_total, 32]
    cos = np.cos(freqs).astype(np.float32)
    sin = np.sin(freqs).astype(np.float32)
    ident = np.eye(d, dtype=np.float32)

    hs = np.asarray(hidden_states, np.float32)
    halves_per_batch = n_cores // hs.shape[0]
    in_maps = []
    for c in range(n_cores):
        b, hf = c // halves_per_batch, c % halves_per_batch
        start = hf * T_main
        chunk = np.empty((H_, T_main + r), np.float32)
        chunk[:, r:] = hs[b, start:start + T_main].T
        if hf == 0:
            chunk[:, :r] = 0.0
        else:
            chunk[:, :r] = hs[b, start - r:start].T
        w0 = hf * nwin
        in_maps.append({
            "hT": _round_f32r(chunk),
            "W": Wr,
            "bias_lo": bias_lo_t,
            "bias_lo0": bias_lo_g0 if hf == 0 else bias_lo_t,
            "bias_hi": bias_hi_t,
            "cos": np.ascontiguousarray(cos[w0:w0 + nwin]),
            "sin": np.ascontiguousarray(sin[w0:w0 + nwin]),
            "ident": ident,
        })
    return in_maps


def kernel(hidden_states, w_kv, w_gate, position_bias, _want_profile=False):
    """Full-input, full-output entry point.  Shards over 8 NeuronCores."""
    from concourse.bass_utils import run_bass_kernel_spmd

    hs = np.asarray(hidden_states, np.float32)
    B_, S_, H_ = hs.shape
    n = S_ // RATIO
    if "nc" not in _CACHE:
        _CACHE["nc"] = build_program(HALF, H_, NWIN_CORE)
    nc = _CACHE["nc"]

    in_maps = _host_inputs(hs, w_kv, w_gate, position_bias,
                           HALF, NWIN_CORE, N_CORES)
    kwargs = {}
    if _want_profile:
        import os

        os.makedirs("work/prof", exist_ok=True)
        kwargs = {"trace": True, "tmpdir": os.path.abspath("work/prof")}
    res = run_bass_kernel_spmd(nc, in_maps, list(range(N_CORES)), **kwargs)

    out = np.empty((B_, n, HEAD_DIM), np.float32)
    halves_per_batch = N_CORES // B_
    for c in range(N_CORES):
        b, hf = c // halves_per_batch, c % halves_per_batch
        out[b, hf * NWIN_CORE:(hf + 1) * NWIN_CORE] = res.results[c]["out"]
    if _want_profile:
        return out, res
    return out


# revision 12
# speedup vs baseline: 1.5142x; 1.0202x over previous
"""Trainium2 Bass kernel: DeepseekV4 CSA Compressor.

Math (per batch b):
  kv = hidden @ w_kv, gate = hidden @ w_gate          [S, 256]
  windows w = 0..S/32-1: tokens [w*32-32, w*32+32)  (prev block -> lo
  channels, current block -> hi channels; window 0 prev = 0 kv / -1e9 gate)
  pooled[w] = sum_j softmax_j(win_g + pos_bias)[j, d] * win_kv[j, d]
  RoPE on trailing 64 dims at position w*32.

Sharding: 8 cores = (4 batches) x (2 sequence halves).  Each core gets its
4096-token chunk transposed on host ([H, 4128] with a 32-token halo column
block in front; zeros for the first half, so the -1e9 gate fill is applied
via a per-core bias variant on the first window group).  No collectives.

Matmuls run in float32r (fp32 with 11-bit mantissa, TF32-like, 4x faster
than fp32 on the PE).  Inputs are pre-rounded to f32r on host.
"""

import numpy as np

HEAD_DIM = 128
ROPE_DIM = 64
RATIO = 32
ROPE_THETA = 10000.0
NEG = -1e9

B, S, H = 4, 8192, 4096
N_CORES = 8
HALF = S // 2                 # tokens per core
NWIN_CORE = HALF // RATIO     # windows per core = 128
GW = 512                      # tokens per matmul/pooling group
WPG = GW // RATIO             # windows per group = 16

_CACHE: dict = {}


def _round_f32r(x: np.ndarray) -> np.ndarray:
    """Round fp32 to f32r (11-bit mantissa, round-to-nearest-even)."""
    u = np.ascontiguousarray(x, dtype=np.float32).view(np.uint32)
    r = (u + np.uint32(0x7FF) + ((u >> np.uint32(12)) & np.uint32(1))) & np.uint32(
        0xFFFFF000
    )
    return r.view(np.float32)


def build_program(T_main: int, H_: int, nwin: int):
    """Build the single-core SPMD Bass program.

    T_main: tokens per core (multiple of GW); H_: hidden dim (multiple of
    128); nwin: windows per core (= T_main // RATIO, multiple of WPG).
    """
    from contextlib import ExitStack

    import concourse.bacc as bacc
    import concourse.mybir as mybir
    import concourse.tile as tile

    f32 = mybir.dt.float32
    f32r = mybir.dt.float32r
    AF = mybir.ActivationFunctionType
    AX = mybir.AxisListType

    d = HEAD_DIM
    r = RATIO
    NG = T_main // GW         # number of groups
    KT = H_ // 128            # k tiles
    C = 4 * d                 # 512 projection channels (kv_lo|kv_hi|g_lo|g_hi)
    NPAIR = NG // 2
    KKT = KT // 2
    PW = 2 * GW + r           # 1056 columns per pair load

    nc = bacc.Bacc("TRN2", target_bir_lowering=False, debug=False,
                   num_devices=N_CORES)
    # Pre-tiled on host in exact consumption order: each [128, 2, PW] block
    # is one fully-contiguous 1.08MB DMA (sequential HBM streaming).
    hTp = nc.dram_tensor("hTp", [NPAIR, KKT, 128, 2, PW], f32r,
                         kind="ExternalInput").ap()
    Wt = nc.dram_tensor("W", [H_, C], f32r, kind="ExternalInput").ap()
    bias_lo = nc.dram_tensor("bias_lo", [d, GW], f32, kind="ExternalInput").ap()
    bias_lo0 = nc.dram_tensor("bias_lo0", [d, GW], f32, kind="ExternalInput").ap()
    bias_hi = nc.dram_tensor("bias_hi", [d, GW], f32, kind="ExternalInput").ap()
    cos_in = nc.dram_tensor("cos", [nwin, ROPE_DIM // 2], f32,
                            kind="ExternalInput").ap()
    sin_in = nc.dram_tensor("sin", [nwin, ROPE_DIM // 2], f32,
                            kind="ExternalInput").ap()
    ident = nc.dram_tensor("ident", [d, d], f32, kind="ExternalInput").ap()
    out = nc.dram_tensor("out", [nwin, d], f32, kind="ExternalOutput").ap()

    with tile.TileContext(nc) as tc, ExitStack() as ctx:
        wp = ctx.enter_context(tc.tile_pool(name="wp", bufs=1))
        hp = ctx.enter_context(tc.tile_pool(name="hp", bufs=4))
        pp = ctx.enter_context(tc.tile_pool(name="pp", bufs=2, space="PSUM"))
        sp = ctx.enter_context(tc.tile_pool(name="sp", bufs=2))
        smp = ctx.enter_context(tc.tile_pool(name="smp", bufs=2))
        cp = ctx.enter_context(tc.tile_pool(name="cp", bufs=1))

        # Stationary weights: one tile per k-tile so the first matmuls only
        # wait on their own chunk (ct: 0=kv_lo 1=kv_hi 2=g_lo 3=g_hi).
        # Small/one-time DMAs go on gpsimd (SWDGE) to keep the two HWDGE
        # engines (sync, scalar) free for the hidden-state stream.
        w_sb = []
        for k in range(KT):
            w_k = wp.tile([128, C], f32r, tag=f"w{k}", name=f"w{k}")
            nc.gpsimd.dma_start(w_k[:], Wt[k * 128:(k + 1) * 128, :])
            w_sb.append(w_k)

        blo = cp.tile([d, GW], f32, tag="blo")
        nc.gpsimd.dma_start(blo[:], bias_lo[:])
        blo0 = cp.tile([d, GW], f32, tag="blo0")
        nc.gpsimd.dma_start(blo0[:], bias_lo0[:])
        bhi = cp.tile([d, GW], f32, tag="bhi")
        nc.gpsimd.dma_start(bhi[:], bias_hi[:])

        pooled = cp.tile([d, nwin], f32, tag="pooled")

        def pooling_pair(pair_groups):
            # Softmax-gated pooling, both groups of a pair interleaved so
            # DVE / ACT / GpSimd pipeline across the two chains.
            # No max-subtraction: gate values are O(5), exp is safe, and
            # the -1e9 first-window fill underflows exp to exactly 0.
            # kv banks are copied out first so the PSUM slots free early
            # (the next pair's matmuls wait on them).
            st = {}
            for s_, (g, (kvlo, kvhi, glo, ghi)) in zip("ab", pair_groups):
                t = st[s_] = {}
                t["kvlo"] = sp.tile([d, GW], f32, tag=f"kvlo_{s_}",
                                    name=f"kvlo_{s_}{g}")
                nc.vector.tensor_copy(t["kvlo"][:], kvlo[:])
                t["kvhi"] = sp.tile([d, GW], f32, tag=f"kvhi_{s_}",
                                    name=f"kvhi_{s_}{g}")
                nc.vector.tensor_copy(t["kvhi"][:], kvhi[:])
            for s_, (g, (kvlo, kvhi, glo, ghi)) in zip("ab", pair_groups):
                t = st[s_]
                t["tglo"] = sp.tile([d, GW], f32, tag=f"tglo_{s_}",
                                    name=f"tglo_{s_}{g}")
                nc.vector.tensor_add(t["tglo"][:], glo[:],
                                     (blo0 if g == 0 else blo)[:])
                t["tghi"] = sp.tile([d, GW], f32, tag=f"tghi_{s_}",
                                    name=f"tghi_{s_}{g}")
                nc.vector.tensor_add(t["tghi"][:], ghi[:], bhi[:])
            for s_, (g, _) in zip("ab", pair_groups):
                t = st[s_]
                t["elo"] = sp.tile([d, GW], f32, tag=f"elo_{s_}",
                                   name=f"elo_{s_}{g}")
                nc.scalar.activation(t["elo"][:], t["tglo"][:], AF.Exp)
                t["ehi"] = sp.tile([d, GW], f32, tag=f"ehi_{s_}",
                                   name=f"ehi_{s_}{g}")
                nc.scalar.activation(t["ehi"][:], t["tghi"][:], AF.Exp)
            for s_, (g, _) in zip("ab", pair_groups):
                t = st[s_]
                t["slo"] = smp.tile([d, WPG], f32, tag=f"slo_{s_}",
                                    name=f"slo_{s_}{g}")
                nc.vector.reduce_sum(
                    t["slo"][:], t["elo"][:].rearrange("p (w j) -> p w j", j=r),
                    axis=AX.X)
                nc.vector.reduce_sum(
                    t.setdefault("shi", smp.tile([d, WPG], f32, tag=f"shi_{s_}",
                                                 name=f"shi_{s_}{g}"))[:],
                    t["ehi"][:].rearrange("p (w j) -> p w j", j=r), axis=AX.X)
                t["plo"] = sp.tile([d, GW], f32, tag=f"plo_{s_}",
                                   name=f"plo_{s_}{g}")
                nc.vector.tensor_mul(t["plo"][:], t["elo"][:], t["kvlo"][:])
                t["phi"] = sp.tile([d, GW], f32, tag=f"phi_{s_}",
                                   name=f"phi_{s_}{g}")
                nc.vector.tensor_mul(t["phi"][:], t["ehi"][:], t["kvhi"][:])
            for s_, (g, _) in zip("ab", pair_groups):
                t = st[s_]
                t["sall"] = smp.tile([d, WPG], f32, tag=f"sall_{s_}",
                                     name=f"sall_{s_}{g}")
                nc.gpsimd.tensor_add(t["sall"][:], t["slo"][:], t["shi"][:])
                t["nlo"] = smp.tile([d, WPG], f32, tag=f"nlo_{s_}",
                                    name=f"nlo_{s_}{g}")
                nc.vector.reduce_sum(
                    t["nlo"][:], t["plo"][:].rearrange("p (w j) -> p w j", j=r),
                    axis=AX.X)
                t["nhi"] = smp.tile([d, WPG], f32, tag=f"nhi_{s_}",
                                    name=f"nhi_{s_}{g}")
                nc.vector.reduce_sum(
                    t["nhi"][:], t["phi"][:].rearrange("p (w j) -> p w j", j=r),
                    axis=AX.X)
            for s_, (g, _) in zip("ab", pair_groups):
                t = st[s_]
                t["rs"] = smp.tile([d, WPG], f32, tag=f"rs_{s_}",
                                   name=f"rs_{s_}{g}")
                nc.vector.reciprocal(t["rs"][:], t["sall"][:])
                t["num"] = smp.tile([d, WPG], f32, tag=f"num_{s_}",
                                    name=f"num_{s_}{g}")
                nc.vector.tensor_add(t["num"][:], t["nlo"][:], t["nhi"][:])
            for s_, (g, _) in zip("ab", pair_groups):
                t = st[s_]
                nc.vector.tensor_mul(pooled[:, g * WPG:(g + 1) * WPG],
                                     t["num"][:], t["rs"][:])

        # Groups processed in pairs: one hT DMA per k-tile covers both
        # groups' (overlapping, 32-shifted) column spans -> 4224B DMA
        # lines and half the descriptor-generation work; the weight tile
        # w_sb[:, k, c] feeds both groups' matmuls.
        assert NG % 2 == 0 and KT % 2 == 0
        for p in range(NPAIR):
            g0, g1 = 2 * p, 2 * p + 1
            ps0 = [pp.tile([d, GW], f32, tag=t, name=f"{t}_a{p}")
                   for t in ("kvlo", "kvhi", "glo", "ghi")]
            ps1 = [pp.tile([d, GW], f32, tag=t, name=f"{t}_b{p}")
                   for t in ("kvlo", "kvhi", "glo", "ghi")]
            for kk in range(KKT):
                # One 1.08MB contiguous DMA: 2 k-tiles x both pair groups.
                ht2 = hp.tile([128, 2, PW], f32r, tag="ht")
                dma_eng = nc.sync if kk % 2 == 0 else nc.scalar
                dma_eng.dma_start(ht2[:], hTp[p, kk])
                for j in range(2):
                    k = 2 * kk + j
                    ht_k = ht2[:, j, :]
                    views = [
                        (ps0[0], ht_k[:, 0:GW], 0),
                        (ps0[1], ht_k[:, r:GW + r], 1),
                        (ps0[2], ht_k[:, 0:GW], 2),
                        (ps0[3], ht_k[:, r:GW + r], 3),
                        (ps1[0], ht_k[:, GW:2 * GW], 0),
                        (ps1[1], ht_k[:, GW + r:2 * GW + r], 1),
                        (ps1[2], ht_k[:, GW:2 * GW], 2),
                        (ps1[3], ht_k[:, GW + r:2 * GW + r], 3),
                    ]
                    st, sp_ = (k == 0), (k == KT - 1)
                    for psum_t, rhs_v, ct in views:
                        nc.tensor.matmul(psum_t[:],
                                         w_sb[k][:, ct * d:(ct + 1) * d],
                                         rhs_v, start=st, stop=sp_)
            pooling_pair([(g0, ps0), (g1, ps1)])

        # Transpose pooled [d, nwin] -> [nwin, d] via PE, then RoPE.
        idt = cp.tile([d, d], f32, tag="idt")
        nc.sync.dma_start(idt[:], ident[:])
        ptr = pp.tile([nwin, d], f32, tag="kvlo")  # reuse a psum slot
        nc.tensor.transpose(ptr[:], pooled[:], idt[:])

        cosb = cp.tile([nwin, ROPE_DIM // 2], f32, tag="cosb")
        nc.sync.dma_start(cosb[:], cos_in[:])
        sinb = cp.tile([nwin, ROPE_DIM // 2], f32, tag="sinb")
        nc.sync.dma_start(sinb[:], sin_in[:])

        outsb = cp.tile([nwin, d], f32, tag="outsb")
        nope_w = d - ROPE_DIM
        nc.vector.tensor_copy(outsb[:, 0:nope_w], ptr[:, 0:nope_w])
        rp = ptr[:, nope_w:d].rearrange("p (a two) -> p a two", two=2)
        re_, ro_ = rp[:, :, 0], rp[:, :, 1]
        op = outsb[:, nope_w:d].rearrange("p (a two) -> p a two", two=2)
        oe_, oo_ = op[:, :, 0], op[:, :, 1]
        hw_ = ROPE_DIM // 2
        t1 = smp.tile([nwin, hw_], f32, tag="t1")
        t2 = smp.tile([nwin, hw_], f32, tag="t2")
        nc.vector.tensor_mul(t1[:], re_, cosb[:])
        nc.vector.tensor_mul(t2[:], ro_, sinb[:])
        nc.vector.tensor_sub(oe_, t1[:], t2[:])
        t3 = smp.tile([nwin, hw_], f32, tag="t3")
        t4 = smp.tile([nwin, hw_], f32, tag="t4")
        nc.vector.tensor_mul(t3[:], ro_, cosb[:])
        nc.vector.tensor_mul(t4[:], re_, sinb[:])
        nc.vector.tensor_add(oo_, t3[:], t4[:])

        nc.sync.dma_start(out[:], outsb[:])

    nc.compile()
    return nc


def _host_inputs(hidden_states, w_kv, w_gate, position_bias,
                 T_main: int, nwin: int, n_cores: int):
    """Build per-core input maps (list of dicts) for the SPMD program."""
    d, r = HEAD_DIM, RATIO
    H_ = hidden_states.shape[2]
    n_total = nwin * n_cores // hidden_states.shape[0]  # windows per batch

    Wfull = np.concatenate([np.asarray(w_kv, np.float32),
                            np.asarray(w_gate, np.float32)], axis=1)
    Wr = _round_f32r(Wfull)

    biasT = np.ascontiguousarray(np.asarray(position_bias, np.float32).T)  # [d, 2r]
    bias_lo_t = np.ascontiguousarray(np.tile(biasT[:, :r], (1, WPG)))
    bias_hi_t = np.ascontiguousarray(np.tile(biasT[:, r:], (1, WPG)))
    bias_lo_g0 = bias_lo_t.copy()
    bias_lo_g0[:, :r] = NEG

    positions = np.arange(n_total, dtype=np.float32) * r
    inv_freq = 1.0 / (ROPE_THETA ** (
        np.arange(0, ROPE_DIM, 2, dtype=np.float32) / ROPE_DIM))
    freqs = positions[:, None] * inv_freq[None, :]         # [n_total, 32]
    cos = np.cos(freqs).astype(np.float32)
    sin = np.sin(freqs).astype(np.float32)
    ident = np.eye(d, dtype=np.float32)

    hs = np.asarray(hidden_states, np.float32)
    halves_per_batch = n_cores // hs.shape[0]
    NPAIR = T_main // (2 * GW)
    KKT = H_ // 256
    PW = 2 * GW + r
    in_maps = []
    for c in range(n_cores):
        b, hf = c // halves_per_batch, c % halves_per_batch
        start = hf * T_main
        chunk = np.empty((H_, T_main + r), np.float32)
        chunk[:, r:] = hs[b, start:start + T_main].T
        if hf == 0:
            chunk[:, :r] = 0.0
        else:
            chunk[:, :r] = hs[b, start - r:start].T
        chunk = _round_f32r(chunk)
        # Pre-tile into exact DMA consumption order:
        # hTp[pair, kk, p, j, c] = chunk[(2kk+j)*128 + p, pair*1024 + c]
        v = chunk.reshape(KKT, 2, 128, T_main + r)
        hTp = np.ascontiguousarray(
            np.stack([v[:, :, :, p0 * 2 * GW:p0 * 2 * GW + PW]
                      for p0 in range(NPAIR)], axis=0).transpose(0, 1, 3, 2, 4))
        w0 = hf * nwin
        in_maps.append({
            "hTp": hTp,
            "W": Wr,
            "bias_lo": bias_lo_t,
            "bias_lo0": bias_lo_g0 if hf == 0 else bias_lo_t,
            "bias_hi": bias_hi_t,
            "cos": np.ascontiguousarray(cos[w0:w0 + nwin]),
            "sin": np.ascontiguousarray(sin[w0:w0 + nwin]),
            "ident": ident,
        })
    return in_maps


def kernel(hidden_states, w_kv, w_gate, position_bias, _want_profile=False):
    """Full-input, full-output entry point.  Shards over 8 NeuronCores."""
    from concourse.bass_utils import run_bass_kernel_spmd

    hs = np.asarray(hidden_states, np.float32)
    B_, S_, H_ = hs.shape
    n = S_ // RATIO
    if "nc" not in _CACHE:
        _CACHE["nc"] = build_program(HALF, H_, NWIN_CORE)
    nc = _CACHE["nc"]

    in_maps = _host_inputs(hs, w_kv, w_gate, position_bias,
                           HALF, NWIN_CORE, N_CORES)
    kwargs = {}
    if _want_profile:
        import os

        os.makedirs("work/prof", exist_ok=True)
        kwargs = {"trace": True, "tmpdir": os.path.abspath("work/prof")}
    res = run_bass_kernel_spmd(nc, in_maps, list(range(N_CORES)), **kwargs)

    out = np.empty((B_, n, HEAD_DIM), np.float32)
    halves_per_batch = N_CORES // B_
    for c in range(N_CORES):
        b, hf = c // halves_per_batch, c % halves_per_batch
        out[b, hf * NWIN_CORE:(hf + 1) * NWIN_CORE] = res.results[c]["out"]
    if _want_profile:
        return out, res
    return out
